# revision 4
# baseline (speedup 1.0000x reference)
"""Trainium2 Bass kernel for the nn_GAT problem (2-layer GAT, N=4096, H=8).

Key structural property exploited: the GAT attention score is
    score[h,i,j] = a_i[h]@x_i + a_j[h]@x_j + b[h]
The softmax is over j, and the i-dependent term (and bias) are constant
along j, so they cancel exactly: attention weights are IDENTICAL for every
query node i.  Hence

    out[i,:] = mean_h( softmax_j(s_j[h,:]) @ e1 )       (same row for all i)

and layer 2 (whose input rows are then all identical) reduces to a uniform
average, i.e. the identity on its (constant) input rows.  The whole network
collapses to one softmax-weighted global mean over nodes + two tiny linears.
This is exact in exact arithmetic for ANY input values (it is a property of
the module, not the data); numerically it matches the fp32 reference to
~5e-5 relative.

Device computation (replicated on all 8 cores, SPMD):
    xt_ext  [65,4096] = [x^T ; ones]                      (host prep)
    Wcomb   [65,73]   = [ones-col | (W1^T;b1) | (W1^T;b1)@a1w_j^T]
    comb    [4096,73] = xt_ext^T @ Wcomb  = [1 | e1 | sj]   (32 PE matmuls)
    p       [4096,8]  = exp(sj)                             (ACT)
    v_ext   [8,65]    = p^T @ comb[:,0:65] = [sum_exp | v]  (32 accum matmuls)
    r1      [64]      = mean_h v[h,:]/s[h]; leaky; elu
    row     [32]      = leaky(W2 @ r1 + b2)
Output = row broadcast to [4096,32] (host).
"""

import sys

import numpy as np

for _p in ("/opt/trn_rl_repo",):
    if _p not in sys.path:
        sys.path.insert(0, _p)

N = 4096
CHUNK = 128
NCHUNK = N // CHUNK  # 32
KC = 65  # contraction: 64 features + ones row
WC = 1 + 64 + 8  # comb cols: ones | e1 | sj
GROUP = 4  # mm1 chunks per psum bank
NCORES = 8

_BUILT = {}


def _build_module():
    import concourse.bass as bass
    import concourse.mybir as mybir
    from concourse import bacc, tile

    fp32 = mybir.dt.float32

    nc = bacc.Bacc(
        "TRN2",
        target_bir_lowering=False,
        debug=False,
        num_devices=NCORES,
    )

    xt_d = nc.dram_tensor("xt_ext", [KC, N], fp32, kind="ExternalInput")
    wcomb_d = nc.dram_tensor("wcomb", [KC, WC], fp32, kind="ExternalInput")
    w2_d = nc.dram_tensor("w2ext", [KC, 32], fp32, kind="ExternalInput")
    out_d = nc.dram_tensor("row_out", [32, 1], fp32, kind="ExternalOutput")

    with tile.TileContext(nc) as tc:
        with (
            tc.tile_pool(name="const", bufs=1) as const_pool,
            tc.tile_pool(name="xt", bufs=1) as xt_pool,
            tc.tile_pool(name="work", bufs=1) as work_pool,
            tc.tile_pool(name="mm1", bufs=4, space="PSUM") as mm1_pool,
            tc.tile_pool(name="acc", bufs=1, space="PSUM") as acc_pool,
            tc.tile_pool(name="tail", bufs=1, space="PSUM") as tail_pool,
        ):
            wcomb_sb = const_pool.tile([KC, WC], fp32)
            w2_sb = const_pool.tile([KC, 32], fp32)
            eighth_sb = const_pool.tile([8, 1], fp32)
            nc.sync.dma_start(wcomb_sb[:], wcomb_d[:])
            nc.sync.dma_start(w2_sb[:], w2_d[:])
            nc.vector.memset(eighth_sb[:], 0.125)

            # x^T (+ones row) streamed in 8 column-slices for DMA parallelism
            xt_sb = xt_pool.tile([KC, N], fp32)
            for k in range(8):
                sl = slice(k * 512, (k + 1) * 512)
                nc.sync.dma_start(xt_sb[:, sl], xt_d[:, sl])

            comb_sb = work_pool.tile([128, NCHUNK * KC], fp32)  # [1|e1] cols per chunk
            p_sb = work_pool.tile([128, NCHUNK * 8], fp32)  # exp(sj) per chunk
            v_ext = acc_pool.tile([8, KC], fp32)

            for g in range(NCHUNK // GROUP):  # 8 groups of 4 chunks
                mm1 = mm1_pool.tile([128, GROUP * WC], fp32)
                for c in range(GROUP):
                    n = g * GROUP + c
                    nc.tensor.matmul(
                        mm1[:, c * WC : (c + 1) * WC],
                        xt_sb[:, n * CHUNK : (n + 1) * CHUNK],
                        wcomb_sb[:],
                        start=True,
                        stop=True,
                    )
                # copy [1|e1] columns (0:65 of each chunk) psum -> sbuf
                nc.vector.tensor_copy(
                    comb_sb[:, g * GROUP * KC : (g + 1) * GROUP * KC].rearrange(
                        "p (n w) -> p n w", w=KC
                    ),
                    mm1[:].rearrange("p (n w) -> p n w", w=WC)[:, :, 0:KC],
                )
                # exp(sj): cols 65:73 of each chunk, psum -> sbuf
                nc.scalar.activation(
                    p_sb[:, g * GROUP * 8 : (g + 1) * GROUP * 8].rearrange(
                        "p (n w) -> p n w", w=8
                    ),
                    mm1[:].rearrange("p (n w) -> p n w", w=WC)[:, :, KC:WC],
                    mybir.ActivationFunctionType.Exp,
                )
                for c in range(GROUP):
                    n = g * GROUP + c
                    nc.tensor.matmul(
                        v_ext[:],
                        p_sb[:, n * 8 : (n + 1) * 8],
                        comb_sb[:, n * KC : (n + 1) * KC],
                        start=(n == 0),
                        stop=(n == NCHUNK - 1),
                    )

            # ---- tail: r1 = mean_h v[h,:]/s[h] ; leaky ; elu ; W2 ; leaky ----
            v_sb = work_pool.tile([8, KC], fp32)
            nc.vector.tensor_copy(v_sb[:], v_ext[:])
            inv_s = work_pool.tile([8, 1], fp32)
            nc.vector.reciprocal(inv_s[:], v_sb[:, 0:1])
            v1n = work_pool.tile([8, 64], fp32)
            nc.vector.tensor_scalar_mul(v1n[:], v_sb[:, 1:KC], inv_s[:])

            r1_ps = tail_pool.tile([64, 1], fp32, tag="tailps")
            nc.tensor.matmul(r1_ps[:], v1n[:], eighth_sb[:], start=True, stop=True)

            # r1_ext[0:64] = elu(leaky(r1)); r1_ext[64] = 1.0
            r1_ext = work_pool.tile([KC, 1], fp32)
            t02 = work_pool.tile([64, 1], fp32)
            lk = work_pool.tile([64, 1], fp32)
            mn = work_pool.tile([64, 1], fp32)
            ex = work_pool.tile([64, 1], fp32)
            rl = work_pool.tile([64, 1], fp32)
            # leaky(x) = max(x, 0.2*x)
            nc.vector.tensor_scalar_mul(t02[:], r1_ps[:], 0.2)
            nc.vector.tensor_tensor(
                lk[:], r1_ps[:], t02[:], op=mybir.AluOpType.max
            )
            # elu(x) = max(x,0) + exp(min(x,0)) - 1
            nc.vector.tensor_scalar_min(mn[:], lk[:], 0.0)
            nc.scalar.activation(ex[:], mn[:], mybir.ActivationFunctionType.Exp)
            nc.vector.tensor_scalar_max(rl[:], lk[:], 0.0)
            nc.vector.tensor_tensor(
                r1_ext[0:64, :], rl[:], ex[:], op=mybir.AluOpType.add
            )
            nc.vector.tensor_scalar_add(r1_ext[0:64, :], r1_ext[0:64, :], -1.0)
            nc.vector.memset(r1_ext[64:65, :], 1.0)

            r2_ps = tail_pool.tile([32, 1], fp32, tag="tailps")
            nc.tensor.matmul(r2_ps[:], w2_sb[:], r1_ext[:], start=True, stop=True)

            out_sb = work_pool.tile([32, 1], fp32)
            t2 = work_pool.tile([32, 1], fp32)
            nc.vector.tensor_scalar_mul(t2[:], r2_ps[:], 0.2)
            nc.vector.tensor_tensor(
                out_sb[:], r2_ps[:], t2[:], op=mybir.AluOpType.max
            )
            nc.sync.dma_start(out_d[:], out_sb[:])

    nc.compile()
    return nc


def _get_module():
    if "nc" not in _BUILT:
        _BUILT["nc"] = _build_module()
    return _BUILT["nc"]


def _host_prep(x, W1, b1, a1_w, W2, b2):
    f32 = np.float32
    x = np.ascontiguousarray(x, f32)
    W1T_ext = np.concatenate([W1.T, b1[None, :]], 0).astype(f32)  # [65,64]
    Wsj = (W1T_ext @ a1_w[:, 64:].T).astype(f32)  # [65,8]
    onescol = np.zeros((KC, 1), f32)
    onescol[64, 0] = 1.0
    Wcomb = np.concatenate([onescol, W1T_ext, Wsj], 1).astype(f32)  # [65,73]
    W2T_ext = np.concatenate([W2.T, b2[None, :]], 0).astype(f32)  # [65,32]
    xt_ext = np.empty((KC, N), f32)
    xt_ext[:64] = x.T
    xt_ext[64] = 1.0
    return xt_ext, Wcomb, W2T_ext


def kernel(x, W1, b1, a1_w, a1_b, W2, b2, a2_w, a2_b):
    from concourse.bass_utils import run_bass_kernel_spmd

    nc = _get_module()
    xt_ext, Wcomb, W2T_ext = _host_prep(x, W1, b1, a1_w, W2, b2)
    in_map = {"xt_ext": xt_ext, "wcomb": Wcomb, "w2ext": W2T_ext}
    res = run_bass_kernel_spmd(nc, [in_map] * NCORES, list(range(NCORES)))
    row = np.asarray(res.results[0]["row_out"], dtype=np.float32).reshape(32)
    out = np.empty((N, 32), np.float32)
    out[:] = row[None, :]
    return out


if __name__ == "__main__":
    rng = np.random.default_rng(0)
    s = lambda f: 1.0 / np.sqrt(f)
    ins = dict(
        x=rng.standard_normal((N, 64)).astype(np.float32),
        W1=(rng.standard_normal((64, 64)) * s(64)).astype(np.float32),
        b1=(rng.standard_normal(64) * s(64)).astype(np.float32),
        a1_w=(rng.standard_normal((8, 128)) * s(128)).astype(np.float32),
        a1_b=(rng.standard_normal(8) * s(128)).astype(np.float32),
        W2=(rng.standard_normal((32, 64)) * s(64)).astype(np.float32),
        b2=(rng.standard_normal(32) * s(64)).astype(np.float32),
        a2_w=(rng.standard_normal((8, 64)) * s(64)).astype(np.float32),
        a2_b=(rng.standard_normal(8) * s(64)).astype(np.float32),
    )
    out = kernel(**ins)
    print("kernel output", out.shape, out.dtype, out[0, :5])


# revision 6
# speedup vs baseline: 1.0594x; 1.0594x over previous
"""Trainium2 Bass kernel for the nn_GAT problem (2-layer GAT, N=4096, H=8).

Key structural property exploited: the GAT attention score is
    score[h,i,j] = a_i[h]@x_i + a_j[h]@x_j + b[h]
The softmax is over j, and the i-dependent term (and bias) are constant
along j, so they cancel exactly: attention weights are IDENTICAL for every
query node i.  Hence

    out[i,:] = mean_h( softmax_j(s_j[h,:]) @ e1 )       (same row for all i)

and layer 2 (whose input rows are then all identical) reduces to a uniform
average, i.e. the identity on its (constant) input rows.  The whole network
collapses to one softmax-weighted global mean over nodes + two tiny linears.
This is exact in exact arithmetic for ANY input values (it is a property of
the module, not the data); numerically it matches the fp32 reference to
~5e-5 relative.

Device computation (replicated on all 8 cores, SPMD):
    xt_ext  [65,4096] = [x^T ; ones]                      (host prep)
    Wcomb   [65,73]   = [ones-col | (W1^T;b1) | (W1^T;b1)@a1w_j^T]
    comb    [4096,73] = xt_ext^T @ Wcomb  = [1 | e1 | sj]   (32 PE matmuls)
    p       [4096,8]  = exp(sj)                             (ACT)
    v_ext   [8,65]    = p^T @ comb[:,0:65] = [sum_exp | v]  (32 accum matmuls)
    r1      [64]      = mean_h v[h,:]/s[h]; leaky; elu
    row     [32]      = leaky(W2 @ r1 + b2)
Output = row broadcast to [4096,32] (host).
"""

import sys

import numpy as np

for _p in ("/opt/trn_rl_repo",):
    if _p not in sys.path:
        sys.path.insert(0, _p)

N = 4096
CHUNK = 128
NCHUNK = N // CHUNK  # 32
KC = 65  # contraction: 64 features + ones row
WC = 1 + 64 + 8  # comb cols: ones | e1 | sj
GROUP = 4  # mm1 chunks per psum bank
NCORES = 8

_BUILT = {}


def _build_module():
    import concourse.bass as bass
    import concourse.mybir as mybir
    from concourse import bacc, tile

    fp32 = mybir.dt.float32

    nc = bacc.Bacc(
        "TRN2",
        target_bir_lowering=False,
        debug=False,
        num_devices=NCORES,
    )

    xt_d = nc.dram_tensor("xt_ext", [KC, N], fp32, kind="ExternalInput")
    wcomb_d = nc.dram_tensor("wcomb", [KC, WC], fp32, kind="ExternalInput")
    w2_d = nc.dram_tensor("w2ext", [KC, 32], fp32, kind="ExternalInput")
    out_d = nc.dram_tensor("row_out", [32, 1], fp32, kind="ExternalOutput")

    with tile.TileContext(nc) as tc:
        with (
            tc.tile_pool(name="const", bufs=1) as const_pool,
            tc.tile_pool(name="xt", bufs=1) as xt_pool,
            tc.tile_pool(name="work", bufs=1) as work_pool,
            tc.tile_pool(name="mm1", bufs=4, space="PSUM") as mm1_pool,
            tc.tile_pool(name="acc", bufs=1, space="PSUM") as acc_pool,
            tc.tile_pool(name="tail", bufs=1, space="PSUM") as tail_pool,
        ):
            wcomb_sb = const_pool.tile([KC, WC], fp32)
            w2_sb = const_pool.tile([KC, 32], fp32)
            eighth_sb = const_pool.tile([8, 1], fp32)
            r1_ext = const_pool.tile([KC, 1], fp32)
            # weights via gpsimd (SWDGE) to keep the HWDGE issue queues free
            nc.gpsimd.dma_start(wcomb_sb[:], wcomb_d[:])
            nc.gpsimd.dma_start(w2_sb[:], w2_d[:])
            nc.vector.memset(eighth_sb[:], 0.125)
            nc.vector.memset(r1_ext[64:65, :], 1.0)

            # x^T (+ones row) in 8 column-slices, issue split across the two
            # HWDGE-capable engines (SP + ACT) since descriptor generation
            # serializes per issuing engine
            xt_sb = xt_pool.tile([KC, N], fp32)
            for k in range(8):
                sl = slice(k * 512, (k + 1) * 512)
                eng = nc.sync if k % 2 == 0 else nc.scalar
                eng.dma_start(xt_sb[:, sl], xt_d[:, sl])

            comb_sb = work_pool.tile([128, NCHUNK * KC], fp32)  # [1|e1] cols per chunk
            p_sb = work_pool.tile([128, NCHUNK * 8], fp32)  # exp(sj) per chunk
            v_ext = acc_pool.tile([8, KC], fp32)

            for g in range(NCHUNK // GROUP):  # 8 groups of 4 chunks
                mm1 = mm1_pool.tile([128, GROUP * WC], fp32)
                for c in range(GROUP):
                    n = g * GROUP + c
                    nc.tensor.matmul(
                        mm1[:, c * WC : (c + 1) * WC],
                        xt_sb[:, n * CHUNK : (n + 1) * CHUNK],
                        wcomb_sb[:],
                        start=True,
                        stop=True,
                    )
                # copy [1|e1] columns (0:65 of each chunk) psum -> sbuf
                nc.vector.tensor_copy(
                    comb_sb[:, g * GROUP * KC : (g + 1) * GROUP * KC].rearrange(
                        "p (n w) -> p n w", w=KC
                    ),
                    mm1[:].rearrange("p (n w) -> p n w", w=WC)[:, :, 0:KC],
                )
                # exp(sj): cols 65:73 of each chunk, psum -> sbuf
                nc.scalar.activation(
                    p_sb[:, g * GROUP * 8 : (g + 1) * GROUP * 8].rearrange(
                        "p (n w) -> p n w", w=8
                    ),
                    mm1[:].rearrange("p (n w) -> p n w", w=WC)[:, :, KC:WC],
                    mybir.ActivationFunctionType.Exp,
                )
                for c in range(GROUP):
                    n = g * GROUP + c
                    nc.tensor.matmul(
                        v_ext[:],
                        p_sb[:, n * 8 : (n + 1) * 8],
                        comb_sb[:, n * KC : (n + 1) * KC],
                        start=(n == 0),
                        stop=(n == NCHUNK - 1),
                    )

            # ---- tail: r1 = mean_h v[h,:]/s[h] ; leaky ; elu ; W2 ; leaky ----
            inv_s = work_pool.tile([8, 1], fp32)
            nc.vector.reciprocal(inv_s[:], v_ext[:, 0:1])  # DVE reads PSUM
            v1n = work_pool.tile([8, 64], fp32)
            nc.vector.tensor_scalar_mul(v1n[:], v_ext[:, 1:KC], inv_s[:])

            r1_ps = tail_pool.tile([64, 1], fp32, tag="tailps")
            nc.tensor.matmul(r1_ps[:], v1n[:], eighth_sb[:], start=True, stop=True)

            # r1_ext[0:64] = elu(leaky(r1)); r1_ext[64] = 1.0 (memset at start)
            t02 = work_pool.tile([64, 1], fp32)
            lk = work_pool.tile([64, 1], fp32)
            mn = work_pool.tile([64, 1], fp32)
            ex = work_pool.tile([64, 1], fp32)
            rl1 = work_pool.tile([64, 1], fp32)
            # leaky(x) = max(x, 0.2*x)
            nc.vector.tensor_scalar_mul(t02[:], r1_ps[:], 0.2)
            nc.vector.tensor_tensor(
                lk[:], r1_ps[:], t02[:], op=mybir.AluOpType.max
            )
            # elu(x) = (max(x,0) - 1) + exp(min(x,0))
            nc.vector.tensor_scalar_min(mn[:], lk[:], 0.0)
            nc.scalar.activation(ex[:], mn[:], mybir.ActivationFunctionType.Exp)
            nc.vector.tensor_scalar(
                rl1[:], lk[:], 0.0, -1.0,
                op0=mybir.AluOpType.max, op1=mybir.AluOpType.add,
            )
            nc.vector.tensor_tensor(
                r1_ext[0:64, :], rl1[:], ex[:], op=mybir.AluOpType.add
            )

            r2_ps = tail_pool.tile([32, 1], fp32, tag="tailps")
            nc.tensor.matmul(r2_ps[:], w2_sb[:], r1_ext[:], start=True, stop=True)

            out_sb = work_pool.tile([32, 1], fp32)
            t2 = work_pool.tile([32, 1], fp32)
            nc.vector.tensor_scalar_mul(t2[:], r2_ps[:], 0.2)
            nc.vector.tensor_tensor(
                out_sb[:], r2_ps[:], t2[:], op=mybir.AluOpType.max
            )
            nc.sync.dma_start(out_d[:], out_sb[:])

    nc.compile()
    return nc


def _get_module():
    if "nc" not in _BUILT:
        _BUILT["nc"] = _build_module()
    return _BUILT["nc"]


def _host_prep(x, W1, b1, a1_w, W2, b2):
    f32 = np.float32
    x = np.ascontiguousarray(x, f32)
    W1T_ext = np.concatenate([W1.T, b1[None, :]], 0).astype(f32)  # [65,64]
    Wsj = (W1T_ext @ a1_w[:, 64:].T).astype(f32)  # [65,8]
    onescol = np.zeros((KC, 1), f32)
    onescol[64, 0] = 1.0
    Wcomb = np.concatenate([onescol, W1T_ext, Wsj], 1).astype(f32)  # [65,73]
    W2T_ext = np.concatenate([W2.T, b2[None, :]], 0).astype(f32)  # [65,32]
    xt_ext = np.empty((KC, N), f32)
    xt_ext[:64] = x.T
    xt_ext[64] = 1.0
    return xt_ext, Wcomb, W2T_ext


def kernel(x, W1, b1, a1_w, a1_b, W2, b2, a2_w, a2_b):
    from concourse.bass_utils import run_bass_kernel_spmd

    nc = _get_module()
    xt_ext, Wcomb, W2T_ext = _host_prep(x, W1, b1, a1_w, W2, b2)
    in_map = {"xt_ext": xt_ext, "wcomb": Wcomb, "w2ext": W2T_ext}
    res = run_bass_kernel_spmd(nc, [in_map] * NCORES, list(range(NCORES)))
    row = np.asarray(res.results[0]["row_out"], dtype=np.float32).reshape(32)
    out = np.empty((N, 32), np.float32)
    out[:] = row[None, :]
    return out


if __name__ == "__main__":
    rng = np.random.default_rng(0)
    s = lambda f: 1.0 / np.sqrt(f)
    ins = dict(
        x=rng.standard_normal((N, 64)).astype(np.float32),
        W1=(rng.standard_normal((64, 64)) * s(64)).astype(np.float32),
        b1=(rng.standard_normal(64) * s(64)).astype(np.float32),
        a1_w=(rng.standard_normal((8, 128)) * s(128)).astype(np.float32),
        a1_b=(rng.standard_normal(8) * s(128)).astype(np.float32),
        W2=(rng.standard_normal((32, 64)) * s(64)).astype(np.float32),
        b2=(rng.standard_normal(32) * s(64)).astype(np.float32),
        a2_w=(rng.standard_normal((8, 64)) * s(64)).astype(np.float32),
        a2_b=(rng.standard_normal(8) * s(64)).astype(np.float32),
    )
    out = kernel(**ins)
    print("kernel output", out.shape, out.dtype, out[0, :5])


# revision 7
# speedup vs baseline: 10625.0514x; 10029.3988x over previous
"""Trainium2 Bass kernel for the nn_GAT problem (2-layer GAT, N=4096, H=8).

Key structural property exploited: the GAT attention score is
    score[h,i,j] = a_i[h]@x_i + a_j[h]@x_j + b[h]
The softmax is over j, and the i-dependent term (and bias) are constant
along j, so they cancel exactly: attention weights are IDENTICAL for every
query node i.  Hence

    out[i,:] = mean_h( softmax_j(s_j[h,:]) @ e1 )       (same row for all i)

and layer 2 (whose input rows are then all identical) reduces to a uniform
average, i.e. the identity on its (constant) input rows.  The whole network
collapses to one softmax-weighted global mean over nodes + two tiny linears.
This is exact in exact arithmetic for ANY input values (it is a property of
the module, not the data); numerically it matches the fp32 reference to
~5e-5 relative.

Device computation (replicated on all 8 cores, SPMD):
    xt_ext  [65,4096] = [x^T ; ones]                      (host prep)
    Wcomb   [65,73]   = [ones-col | (W1^T;b1) | (W1^T;b1)@a1w_j^T]
    comb    [4096,73] = xt_ext^T @ Wcomb  = [1 | e1 | sj]   (32 PE matmuls)
    p       [4096,8]  = exp(sj)                             (ACT)
    v_ext   [8,65]    = p^T @ comb[:,0:65] = [sum_exp | v]  (32 accum matmuls)
    r1      [64]      = mean_h v[h,:]/s[h]; leaky; elu
    row     [32]      = leaky(W2 @ r1 + b2)
Output = row broadcast to [4096,32] (host).
"""

import sys

import numpy as np

for _p in ("/opt/trn_rl_repo",):
    if _p not in sys.path:
        sys.path.insert(0, _p)

N = 4096
CHUNK = 128
NCHUNK = N // CHUNK  # 32
KC = 65  # contraction: 64 features + ones row
WC = 1 + 64 + 8  # comb cols: ones | e1 | sj
GROUP = 4  # mm1 chunks per psum bank
NCORES = 8

_BUILT = {}


def _build_module():
    import concourse.bass as bass
    import concourse.mybir as mybir
    from concourse import bacc, tile

    fp32 = mybir.dt.float32

    nc = bacc.Bacc(
        "TRN2",
        target_bir_lowering=False,
        debug=False,
        num_devices=NCORES,
    )

    xt_d = nc.dram_tensor("xt_ext", [KC, N], fp32, kind="ExternalInput")
    wcomb_d = nc.dram_tensor("wcomb", [KC, WC], fp32, kind="ExternalInput")
    w2_d = nc.dram_tensor("w2ext", [KC, 32], fp32, kind="ExternalInput")
    out_d = nc.dram_tensor("row_out", [32, 1], fp32, kind="ExternalOutput")

    with tile.TileContext(nc) as tc:
        with (
            tc.tile_pool(name="const", bufs=1) as const_pool,
            tc.tile_pool(name="xt", bufs=1) as xt_pool,
            tc.tile_pool(name="work", bufs=1) as work_pool,
            tc.tile_pool(name="mm1", bufs=4, space="PSUM") as mm1_pool,
            tc.tile_pool(name="acc", bufs=1, space="PSUM") as acc_pool,
            tc.tile_pool(name="tail", bufs=1, space="PSUM") as tail_pool,
        ):
            wcomb_sb = const_pool.tile([KC, WC], fp32)
            w2_sb = const_pool.tile([KC, 32], fp32)
            eighth_sb = const_pool.tile([8, 1], fp32)
            r1_ext = const_pool.tile([KC, 1], fp32)
            # weights via gpsimd (SWDGE) to keep the HWDGE issue queues free
            nc.gpsimd.dma_start(wcomb_sb[:], wcomb_d[:])
            nc.gpsimd.dma_start(w2_sb[:], w2_d[:])
            nc.vector.memset(eighth_sb[:], 0.125)
            nc.vector.memset(r1_ext[64:65, :], 1.0)

            # x^T (+ones row) in 8 column-slices, issue split across the two
            # HWDGE-capable engines (SP + ACT) since descriptor generation
            # serializes per issuing engine
            xt_sb = xt_pool.tile([KC, N], fp32)
            for k in range(8):
                sl = slice(k * 512, (k + 1) * 512)
                eng = nc.sync if k % 2 == 0 else nc.scalar
                eng.dma_start(xt_sb[:, sl], xt_d[:, sl])

            comb_sb = work_pool.tile([128, NCHUNK * KC], fp32)  # [1|e1] cols per chunk
            p_sb = work_pool.tile([128, NCHUNK * 8], fp32)  # exp(sj) per chunk
            v_ext = acc_pool.tile([8, KC], fp32)

            for g in range(NCHUNK // GROUP):  # 8 groups of 4 chunks
                mm1 = mm1_pool.tile([128, GROUP * WC], fp32)
                for c in range(GROUP):
                    n = g * GROUP + c
                    nc.tensor.matmul(
                        mm1[:, c * WC : (c + 1) * WC],
                        xt_sb[:, n * CHUNK : (n + 1) * CHUNK],
                        wcomb_sb[:],
                        start=True,
                        stop=True,
                    )
                # copy [1|e1] columns (0:65 of each chunk) psum -> sbuf
                nc.vector.tensor_copy(
                    comb_sb[:, g * GROUP * KC : (g + 1) * GROUP * KC].rearrange(
                        "p (n w) -> p n w", w=KC
                    ),
                    mm1[:].rearrange("p (n w) -> p n w", w=WC)[:, :, 0:KC],
                )
                # exp(sj): cols 65:73 of each chunk, psum -> sbuf
                nc.scalar.activation(
                    p_sb[:, g * GROUP * 8 : (g + 1) * GROUP * 8].rearrange(
                        "p (n w) -> p n w", w=8
                    ),
                    mm1[:].rearrange("p (n w) -> p n w", w=WC)[:, :, KC:WC],
                    mybir.ActivationFunctionType.Exp,
                )
                for c in range(GROUP):
                    n = g * GROUP + c
                    nc.tensor.matmul(
                        v_ext[:],
                        p_sb[:, n * 8 : (n + 1) * 8],
                        comb_sb[:, n * KC : (n + 1) * KC],
                        start=(n == 0),
                        stop=(n == NCHUNK - 1),
                    )

            # ---- tail: r1 = mean_h v[h,:]/s[h] ; leaky ; elu ; W2 ; leaky ----
            inv_s = work_pool.tile([8, 1], fp32)
            nc.vector.reciprocal(inv_s[:], v_ext[:, 0:1])  # DVE reads PSUM
            v1n = work_pool.tile([8, 64], fp32)
            nc.vector.tensor_scalar_mul(v1n[:], v_ext[:, 1:KC], inv_s[:])

            r1_ps = tail_pool.tile([64, 1], fp32, tag="tailps")
            nc.tensor.matmul(r1_ps[:], v1n[:], eighth_sb[:], start=True, stop=True)

            # r1_ext[0:64] = elu(leaky(r1)); r1_ext[64] = 1.0 (memset at start)
            t02 = work_pool.tile([64, 1], fp32)
            lk = work_pool.tile([64, 1], fp32)
            mn = work_pool.tile([64, 1], fp32)
            ex = work_pool.tile([64, 1], fp32)
            rl1 = work_pool.tile([64, 1], fp32)
            # leaky(x) = max(x, 0.2*x)
            nc.vector.tensor_scalar_mul(t02[:], r1_ps[:], 0.2)
            nc.vector.tensor_tensor(
                lk[:], r1_ps[:], t02[:], op=mybir.AluOpType.max
            )
            # elu(x) = (max(x,0) - 1) + exp(min(x,0))
            nc.vector.tensor_scalar_min(mn[:], lk[:], 0.0)
            nc.scalar.activation(ex[:], mn[:], mybir.ActivationFunctionType.Exp)
            nc.vector.tensor_scalar(
                rl1[:], lk[:], 0.0, -1.0,
                op0=mybir.AluOpType.max, op1=mybir.AluOpType.add,
            )
            nc.vector.tensor_tensor(
                r1_ext[0:64, :], rl1[:], ex[:], op=mybir.AluOpType.add
            )

            r2_ps = tail_pool.tile([32, 1], fp32, tag="tailps")
            nc.tensor.matmul(r2_ps[:], w2_sb[:], r1_ext[:], start=True, stop=True)

            out_sb = work_pool.tile([32, 1], fp32)
            t2 = work_pool.tile([32, 1], fp32)
            nc.vector.tensor_scalar_mul(t2[:], r2_ps[:], 0.2)
            nc.vector.tensor_tensor(
                out_sb[:], r2_ps[:], t2[:], op=mybir.AluOpType.max
            )
            nc.sync.dma_start(out_d[:], out_sb[:])

    nc.compile()
    return nc


def _get_module():
    if "nc" not in _BUILT:
        _BUILT["nc"] = _build_module()
    return _BUILT["nc"]


def _host_prep(x, W1, b1, a1_w, W2, b2):
    f32 = np.float32
    x = np.asarray(x, f32)
    W1 = np.asarray(W1, f32)
    b1 = np.asarray(b1, f32)
    a1_w = np.asarray(a1_w, f32)
    W2 = np.asarray(W2, f32)
    b2 = np.asarray(b2, f32)
    assert x.shape == (N, 64) and W1.shape == (64, 64) and a1_w.shape == (8, 128)
    W1T_ext = np.concatenate([W1.T, b1[None, :]], 0).astype(f32)  # [65,64]
    Wsj = (W1T_ext @ a1_w[:, 64:].T).astype(f32)  # [65,8]
    onescol = np.zeros((KC, 1), f32)
    onescol[64, 0] = 1.0
    Wcomb = np.concatenate([onescol, W1T_ext, Wsj], 1).astype(f32)  # [65,73]
    W2T_ext = np.concatenate([W2.T, b2[None, :]], 0).astype(f32)  # [65,32]
    xt_ext = np.empty((KC, N), f32)
    xt_ext[:64] = x.T
    xt_ext[64] = 1.0
    return xt_ext, Wcomb, W2T_ext


def kernel(x, W1, b1, a1_w, a1_b, W2, b2, a2_w, a2_b):
    from concourse.bass_utils import run_bass_kernel_spmd

    nc = _get_module()
    xt_ext, Wcomb, W2T_ext = _host_prep(x, W1, b1, a1_w, W2, b2)
    in_map = {"xt_ext": xt_ext, "wcomb": Wcomb, "w2ext": W2T_ext}
    res = run_bass_kernel_spmd(nc, [in_map] * NCORES, list(range(NCORES)))
    row = np.asarray(res.results[0]["row_out"], dtype=np.float32).reshape(32)
    out = np.empty((N, 32), np.float32)
    out[:] = row[None, :]
    return out


if __name__ == "__main__":
    rng = np.random.default_rng(0)
    s = lambda f: 1.0 / np.sqrt(f)
    ins = dict(
        x=rng.standard_normal((N, 64)).astype(np.float32),
        W1=(rng.standard_normal((64, 64)) * s(64)).astype(np.float32),
        b1=(rng.standard_normal(64) * s(64)).astype(np.float32),
        a1_w=(rng.standard_normal((8, 128)) * s(128)).astype(np.float32),
        a1_b=(rng.standard_normal(8) * s(128)).astype(np.float32),
        W2=(rng.standard_normal((32, 64)) * s(64)).astype(np.float32),
        b2=(rng.standard_normal(32) * s(64)).astype(np.float32),
        a2_w=(rng.standard_normal((8, 64)) * s(64)).astype(np.float32),
        a2_b=(rng.standard_normal(8) * s(64)).astype(np.float32),
    )
    out = kernel(**ins)
    print("kernel output", out.shape, out.dtype, out[0, :5])
